# revision 74
# baseline (speedup 1.0000x reference)
"""Trainium2 Bass kernel for BestRQ vector-quantization codebook lookup.

Key algebraic move: the projection never materializes on chip.  Scores
satisfy  t.c = (P xhat).c = xhat.(P^T cb) = xhat.G  with G = P^T cb
precomputed on host in fp64 (LN affine folded into P and the bias), so the
contraction shrinks from H=1024 to D=512 and the whole on-chip projection
phase disappears.

Per NeuronCore (data-parallel over batch):
  x (2048,512) --LayerNorm--> xhat --PE transpose--> xnT (d-major)
  x split (2048 scale): xh = fp16(xhat*2048); fp8 pairs xl8 (residual),
  xh8 (fp8(xhat)) in DoubleRow pair layout.
  G split on host: G16 = fp16(G); G8 = fp8(G16); Gl8 = fp8(2048*(G-G16)),
  fp8 parts in DoubleRow pair layout; bias = 2048*(0.5||c||^2 - (P b).c).
  score*2048 = xh@G16  (4 fp16 matmuls, 1 cyc/row)
             + xl8(x)G8 + xh8(x)Gl8  (4 DoubleRow fp8 matmuls, 0.5 cyc/row)
  argmax via two fused custom DVE ops per (1024-col chunk pair,
  token-tile): rowmax(acc - bias) then first-index-equal; the 8-pair
  combine runs inline after each token tile's last pair.

Numerics: G carried to ~2^-15, xhat to ~2^-15; measured on hardware:
1 / 16384 label flips vs the fp32 reference (rel err 8.8e-4, gate 2e-2).

Dispatch-overhead notes: every PJRT argument costs ~56us per call on this
axon setup, so all inputs ship in ONE flat f16 tensor (f32/f8 regions are
read through bitcast views).
"""

import numpy as np

import concourse.bacc as bacc
import concourse.bass as bass
import concourse.mybir as mybir
import concourse.tile as tile
from concourse import dve_ops as _dvo
from concourse.bass_utils import run_bass_kernel_spmd
from concourse.dve_spec import (AluOp as _AluOp, C0 as _C0, C1 as _C1,
                                Idx as _Idx, Spec as _Spec,
                                Src0 as _Src0, Src1 as _Src1, _has_src1,
                                eq as _eq, lower as _lower,
                                select as _select)
from concourse.dve_uop import DveOpSpec as _DveOpSpec
from concourse.masks import make_identity

import ml_dtypes
_F8NP = ml_dtypes.float8_e4m3fn


def _register_dve_op(name, spec):
    """Register a custom fused DVE op (documented extension point: the uop
    program is compiled into the per-NEFF DVE table; sha self-pinned)."""
    for op in _dvo.OPS:
        if op.name == name:
            return op
    op = _dvo.DveOp(name, spec, subdim=False, uops_sha={})
    _dvo.OPS.append(op)
    _dvo.CUSTOM_DVE_SPECS[name] = spec
    opcode = _dvo._CUSTOM_DVE_ROW_BASE + len(_dvo.OPS) - 1
    _dvo._SUB_OPCODE_FOR_NAME[name] = opcode
    for ver in ("v3", "v4"):
        s = _DveOpSpec(name=name, opcode=opcode, uops=_lower(spec, ver=ver),
                       rd1_en=_has_src1(spec))
        op.uops_sha[ver] = s.sha(ver)
    return op


# out = in0 - in1 (elementwise); accum_out = rowmax(out)
_SUB_MAX = _register_dve_op(
    "VQ_SUB_MAX", _Spec(body=_Src0 - _Src1, accum=_AluOp.MAX))
# out = where(in0 - in1 == s0, k, s1); accum_out = rowmin(out)
# (first free-dim index where in0-in1 equals the row max s0)
_IDX_MIN = _register_dve_op(
    "VQ_IDX_MIN", _Spec(body=_select(_eq(_Src0 - _Src1, _C0), _Idx, _C1),
                        accum=_AluOp.MIN, accum_init=_C1))
# out = where(in0 == s0, k, s1); accum_out = rowmin(out)
_IDX_MIN_S = _register_dve_op(
    "VQ_IDX_MIN_S", _Spec(body=_select(_eq(_Src0, _C0), _Idx, _C1),
                          accum=_AluOp.MIN, accum_init=_C1))
# accum_out = min over k of (in1[k] if in0[k] >= s0 else s1)
_PICK_MIN = _register_dve_op(
    "VQ_PICK_MIN", _Spec(body=_select(_Src0 >= _C0, _Src1, _C1),
                         accum=_AluOp.MIN, accum_init=_C1))
# out = in0 * s0 - in1  (split residual in one pass)
_SCALE_SUB = _register_dve_op(
    "VQ_SCALE_SUB", _Spec(body=_Src0 * _C0 - _Src1))

B, L, D, H, C = 8, 2048, 512, 1024, 8192
LN_EPS = 1e-5
N_CORES = 8

TT = L // 128      # 16 token tiles
CCH = C // 512     # 16 codebook chunks
DT = D // 128      # 4 d tiles
DP = DT // 2       # 2 DoubleRow d-tile pairs
TOKC = L // 512    # 4 token chunks
SC = 2048.0        # hi-part scale (exact power of two)

F32 = mybir.dt.float32
F16 = mybir.dt.float16
F8 = mybir.dt.float8e4
I32 = mybir.dt.int32

# packed-input element offsets.  One flat f16 tensor; leading region holds
# f32 data, trailing region holds fp8 pair-layout data (both via bitcast).
OFF_X = 0                        # (L, D) f32        (f32-view offsets)
OFF_BIAS = OFF_X + L * D         # (C,) f32
NF32 = OFF_BIAS + C
OFF_G16 = 2 * NF32               # (D, C) f16        (f16 offsets)
OFF8_G8 = 2 * (OFF_G16 + D * C)  # (DP,128,2,C) f8   (f8/byte offsets)
OFF8_GL8 = OFF8_G8 + D * C       # (DP,128,2,C) f8
NTOT = (OFF8_GL8 + D * C) // 2   # total f16 elements


def build_nc():
    nc = bacc.Bacc("TRN2", target_bir_lowering=False, debug=False)

    d_f16 = nc.dram_tensor("inp", (NTOT,), F16, kind="ExternalInput")
    d_f32 = d_f16.bitcast(F32)
    d_f8 = d_f16.bitcast(F8)
    d_lab = nc.dram_tensor("labels", (128, TT), I32, kind="ExternalOutput")

    with tile.TileContext(nc) as tc:
        with tc.tile_pool(name="consts", bufs=1) as consts, \
             tc.tile_pool(name="persist", bufs=1) as persist:

            # ---------- constants ----------
            ident = consts.tile([128, 128], F32)
            make_identity(nc, ident)
            eps_t = consts.tile([128, 1], F32)
            nc.vector.memset(eps_t, LN_EPS)
            chunk_off = consts.tile([128, CCH // 2], F32)
            for j in range(CCH // 2):
                nc.vector.memset(chunk_off[:, j:j + 1], 1024.0 * j)

            # persistent split of xhat^T (d-major, 2048 scale):
            #   xh16: fp16 hi; x8l: fp8 residual pairs; x8h: fp8(xhat) pairs
            xh16 = [persist.tile([128, L], F16, name=f"xh{d}", tag=f"xh{d}")
                    for d in range(DT)]
            x8l = [persist.tile([128, 2, L], F8, name=f"x8l{j}", tag=f"x8l{j}")
                   for j in range(DP)]
            x8h = [persist.tile([128, 2, L], F8, name=f"x8h{j}", tag=f"x8h{j}")
                   for j in range(DP)]
            # per-token-tile chunk-pair winners (one col per 1024-col pair)
            gval = [persist.tile([128, CCH // 2], F32, name=f"gval{t}",
                                 tag=f"gval{t}") for t in range(TT)]
            gidx = [persist.tile([128, CCH // 2], F32, name=f"gidx{t}",
                                 tag=f"gidx{t}") for t in range(TT)]

            # ---------- phase A: LN + transpose + split ----------
            with tc.tile_pool(name="phA", bufs=1) as phA, \
                 tc.tile_pool(name="ldtmp", bufs=4) as ldtmp, \
                 tc.tile_pool(name="psTr", bufs=4, space="PSUM") as psTr:

                xnT = [phA.tile([128, L], F32, name=f"xnT{d}", tag=f"xnT{d}")
                       for d in range(DT)]
                for t in range(TT):
                    x_t = ldtmp.tile([128, D], F32, tag="x_t")
                    nc.sync.dma_start(
                        out=x_t,
                        in_=bass.AP(tensor=d_f32, offset=OFF_X + t * 128 * D,
                                    ap=[[D, 128], [1, D]]))
                    stats = ldtmp.tile([128, 6], F32, tag="stats")
                    nc.vector.bn_stats(out=stats, in_=x_t)
                    mv = ldtmp.tile([128, 2], F32, tag="mv")
                    nc.vector.bn_aggr(out=mv, in_=stats)
                    rstd = ldtmp.tile([128, 1], F32, tag="rstd")
                    nc.scalar.activation(out=rstd, in_=mv[:, 1:2],
                                         func=mybir.ActivationFunctionType.Sqrt,
                                         bias=eps_t, scale=1.0)
                    nc.vector.reciprocal(out=rstd, in_=rstd)
                    # ln weight/bias are folded into G and the bias on host
                    xn = ldtmp.tile([128, D], F32, tag="xn")
                    nc.vector.tensor_scalar(
                        out=xn, in0=x_t, scalar1=mv[:, 0:1], scalar2=rstd,
                        op0=mybir.AluOpType.subtract, op1=mybir.AluOpType.mult)
                    for d in range(DT):
                        ps_tr = psTr.tile([128, 128], F32, tag="ps_tr")
                        nc.tensor.transpose(ps_tr, xn[:, d * 128:(d + 1) * 128],
                                            ident)
                        nc.scalar.copy(out=xnT[d][:, t * 128:(t + 1) * 128],
                                       in_=ps_tr)
                    if t % 4 == 3:
                        # split this token chunk right away so phase B can
                        # start early; xh16/lo on DVE (idle here) so the ACT
                        # queue stays clear for the remaining drains
                        tksl = slice((t // 4) * 512, (t // 4 + 1) * 512)
                        for d in range(DT):
                            nc.vector.tensor_scalar(
                                out=xh16[d][:, tksl], in0=xnT[d][:, tksl],
                                scalar1=SC, scalar2=None,
                                op0=mybir.AluOpType.mult)
                            nc.scalar.activation(
                                out=x8h[d // 2][:, d % 2, tksl],
                                in_=xh16[d][:, tksl],
                                func=mybir.ActivationFunctionType.Copy,
                                scale=1.0 / SC)
                            nc.vector._custom_dve(
                                _SCALE_SUB, out=x8l[d // 2][:, d % 2, tksl],
                                in0=xnT[d][:, tksl], in1=xh16[d][:, tksl],
                                s0=SC)

            # ---------- phase B: scores + per-chunk argmax (+ inline C) ----
            with tc.tile_pool(name="cbf", bufs=3) as cbf_pool, \
                 tc.tile_pool(name="strips", bufs=2) as strips, \
                 tc.tile_pool(name="fin", bufs=2) as fin, \
                 tc.tile_pool(name="psB", bufs=4, space="PSUM") as psB:

                # chunk PAIRS: matmuls still produce 512-wide groups (one
                # PSUM bank each), but the two banks are adjacent and the
                # fused DVE argmax passes scan 1024 at once (halves the
                # per-instruction overhead on the critical DVE chain).
                NP2 = CCH // 2
                for pp in range(NP2):
                    g16 = []
                    for d in range(DT):
                        t_ = cbf_pool.tile([128, 1024], F16, name=f"g16_{d}",
                                           tag=f"g16_{d}")
                        nc.sync.dma_start(
                            out=t_,
                            in_=bass.AP(tensor=d_f16,
                                        offset=OFF_G16 + d * 128 * C + pp * 1024,
                                        ap=[[C, 128], [1, 1024]]))
                        g16.append(t_)
                    g8, gl8 = [], []
                    for j in range(DP):
                        for lst, base, nm in ((g8, OFF8_G8, "g8"),
                                              (gl8, OFF8_GL8, "gl8")):
                            t_ = cbf_pool.tile([128, 2, 1024], F8,
                                               name=f"{nm}_{j}", tag=f"{nm}_{j}")
                            nc.sync.dma_start(
                                out=t_,
                                in_=bass.AP(tensor=d_f8,
                                            offset=base + j * 256 * C + pp * 1024,
                                            ap=[[2 * C, 128], [C, 2], [1, 1024]]))
                            lst.append(t_)
                    bias_cc = cbf_pool.tile([128, 1024], F32, name="bias_cc",
                                            tag="bias_cc")
                    nc.sync.dma_start(
                        out=bias_cc,
                        in_=bass.AP(tensor=d_f32, offset=OFF_BIAS + pp * 1024,
                                    ap=[[0, 128], [1, 1024]]))

                    for t2 in range(0, TT, 2):
                        accs = []
                        for t in (t2, t2 + 1):
                            tsl = slice(t * 128, (t + 1) * 128)
                            acc = psB.tile([128, 1024], F32, tag="acc")
                            for half in range(2):
                                hsl = slice(half * 512, (half + 1) * 512)
                                for d in range(DT):
                                    nc.tensor.matmul(
                                        acc[:, hsl], lhsT=xh16[d][:, tsl],
                                        rhs=g16[d][:, hsl], start=(d == 0),
                                        stop=False)
                                for j in range(DP):
                                    nc.tensor.matmul(
                                        acc[:, hsl], lhsT=x8l[j][:, :, tsl],
                                        rhs=g8[j][:, :, hsl],
                                        perf_mode=mybir.MatmulPerfMode.DoubleRow,
                                        start=False, stop=False)
                                for j in range(DP):
                                    nc.tensor.matmul(
                                        acc[:, hsl], lhsT=x8h[j][:, :, tsl],
                                        rhs=gl8[j][:, :, hsl],
                                        perf_mode=mybir.MatmulPerfMode.DoubleRow,
                                        start=False, stop=(j == DP - 1))
                            accs.append(acc)
                        # fused DVE, batched by op type to minimize DVE-table
                        # row switches: rowmax(acc - bias) x2, then first-
                        # index-equal x2
                        sjunks = []
                        for t, acc in zip((t2, t2 + 1), accs):
                            junk = strips.tile([128, 1024], F32, tag="junk")
                            nc.vector._custom_dve(
                                _SUB_MAX, out=junk, in0=acc, in1=bias_cc,
                                accum_out=gval[t][:, pp:pp + 1])
                            sjunks.append(junk)
                        for t, sjunk in zip((t2, t2 + 1), sjunks):
                            junk2 = strips.tile([128, 1024], F32, tag="junk2")
                            nc.vector._custom_dve(
                                _IDX_MIN_S, out=junk2, in0=sjunk,
                                s0=gval[t][:, pp:pp + 1], s1=1.0e9,
                                accum_out=gidx[t][:, pp:pp + 1])

                        if pp == NP2 - 1:
                            for t in (t2, t2 + 1):
                                # phase C: combine this token tile's winners
                                gmx = fin.tile([128, 1], F32, tag="gmx")
                                nc.vector.tensor_reduce(
                                    out=gmx, in_=gval[t],
                                    axis=mybir.AxisListType.X,
                                    op=mybir.AluOpType.max)
                                cand = fin.tile([128, NP2], F32, tag="cand")
                                nc.vector.tensor_add(cand, gidx[t], chunk_off)
                                junk3 = fin.tile([128, NP2], F32, tag="junk3")
                                win = fin.tile([128, 1], F32, tag="win")
                                nc.vector._custom_dve(
                                    _PICK_MIN, out=junk3, in0=gval[t],
                                    in1=cand, s0=gmx, s1=1.0e9, accum_out=win)
                                lab = fin.tile([128, 1], I32, tag="lab")
                                nc.vector.tensor_copy(lab, win)
                                nc.sync.dma_start(out=d_lab[:, t:t + 1],
                                                  in_=lab)

    nc.compile()
    return nc


_NC_CACHE = None


def make_in_maps(input_values, ln_weight, ln_bias, proj_weight, codebook):
    input_values = np.ascontiguousarray(input_values, np.float32)
    # Fold the LN affine into the projection, then fold the projection into
    # the codebook:  t.c = xhat.(P'^T cb) with P' = P diag(g); the constant
    # P b shifts every score by (P b).c, folded into the bias (fp64, exact).
    pw64 = proj_weight.astype(np.float64) \
        * np.asarray(ln_weight, np.float64)[None, :]
    pb = proj_weight.astype(np.float64) @ np.asarray(ln_bias, np.float64)
    cb64 = codebook.astype(np.float64)
    G = pw64.T @ cb64                                     # (D, C)
    bias = (SC * (0.5 * (cb64 ** 2).sum(0) - pb @ cb64)).astype(np.float32)

    G16 = G.astype(np.float16)
    G8 = G16.astype(_F8NP)
    Gl8 = ((G - G16.astype(np.float64)) * SC).astype(np.float32).astype(_F8NP)
    # DoubleRow pair layout: [pair j, partition p, slot s, c] where slot s
    # holds d-tile 2j+s (d = (2j+s)*128 + p)
    G8p = np.ascontiguousarray(G8.reshape(DP, 2, 128, C).transpose(0, 2, 1, 3))
    Gl8p = np.ascontiguousarray(
        Gl8.reshape(DP, 2, 128, C).transpose(0, 2, 1, 3))

    tail16 = np.concatenate([
        G16.ravel(),
        G8p.reshape(-1).view(np.uint8).view(np.float16),
        Gl8p.reshape(-1).view(np.uint8).view(np.float16),
    ])

    in_maps = []
    for i in range(N_CORES):
        head32 = np.concatenate([input_values[i].ravel(),
                                 bias.ravel()]).astype(np.float32)
        blob = np.concatenate([head32.view(np.float16), tail16])
        in_maps.append({"inp": np.ascontiguousarray(blob, np.float16)})
    return in_maps


def kernel(input_values, ln_weight, ln_bias, proj_weight, codebook):
    global _NC_CACHE
    if _NC_CACHE is None:
        _NC_CACHE = build_nc()
    nc = _NC_CACHE

    in_maps = make_in_maps(input_values, ln_weight, ln_bias, proj_weight,
                           codebook)
    res = run_bass_kernel_spmd(nc, in_maps, core_ids=list(range(N_CORES)))
    out = np.empty((B, L), np.int32)
    for i in range(N_CORES):
        out[i] = res.results[i]["labels"].T.reshape(L)
    return out


# revision 77
# speedup vs baseline: 1.0215x; 1.0215x over previous
"""Trainium2 Bass kernel for BestRQ vector-quantization codebook lookup.

Key algebraic move: the projection never materializes on chip.  Scores
satisfy  t.c = (P xhat).c = xhat.(P^T cb) = xhat.G  with G = P^T cb
precomputed on host in fp64 (LN affine folded into P and the bias), so the
contraction shrinks from H=1024 to D=512 and the whole on-chip projection
phase disappears.

Per NeuronCore (data-parallel over batch):
  x (2048,512) --LayerNorm--> xhat --PE transpose--> xnT (d-major)
  x split (2048 scale): xh = fp16(xhat*2048); fp8 pairs xl8 (residual),
  xh8 (fp8(xhat)) in DoubleRow pair layout.
  G split on host: G16 = fp16(G); G8 = fp8(G16); Gl8 = fp8(2048*(G-G16)),
  fp8 parts in DoubleRow pair layout; bias = 2048*(0.5||c||^2 - (P b).c).
  score*2048 = xh@G16  (4 fp16 matmuls, 1 cyc/row)
             + xl8(x)G8 + xh8(x)Gl8  (4 DoubleRow fp8 matmuls, 0.5 cyc/row)
  argmax via two fused custom DVE ops per (1024-col chunk pair,
  token-tile): rowmax(acc - bias) then first-index-equal; the 8-pair
  combine runs inline after each token tile's last pair.

Numerics: G carried to ~2^-15, xhat to ~2^-15; measured on hardware:
1 / 16384 label flips vs the fp32 reference (rel err 8.8e-4, gate 2e-2).

Dispatch-overhead notes: every PJRT argument costs ~56us per call on this
axon setup, so all inputs ship in ONE flat f16 tensor (f32/f8 regions are
read through bitcast views).
"""

import numpy as np

import concourse.bacc as bacc
import concourse.bass as bass
import concourse.mybir as mybir
import concourse.tile as tile
from concourse import dve_ops as _dvo
from concourse.bass_utils import run_bass_kernel_spmd
from concourse.dve_spec import (AluOp as _AluOp, C0 as _C0, C1 as _C1,
                                Idx as _Idx, Spec as _Spec,
                                Src0 as _Src0, Src1 as _Src1, _has_src1,
                                eq as _eq, lower as _lower,
                                select as _select)
from concourse.dve_uop import DveOpSpec as _DveOpSpec
from concourse.masks import make_identity

import ml_dtypes
_F8NP = ml_dtypes.float8_e4m3fn


def _register_dve_op(name, spec):
    """Register a custom fused DVE op (documented extension point: the uop
    program is compiled into the per-NEFF DVE table; sha self-pinned)."""
    for op in _dvo.OPS:
        if op.name == name:
            return op
    op = _dvo.DveOp(name, spec, subdim=False, uops_sha={})
    _dvo.OPS.append(op)
    _dvo.CUSTOM_DVE_SPECS[name] = spec
    opcode = _dvo._CUSTOM_DVE_ROW_BASE + len(_dvo.OPS) - 1
    _dvo._SUB_OPCODE_FOR_NAME[name] = opcode
    for ver in ("v3", "v4"):
        s = _DveOpSpec(name=name, opcode=opcode, uops=_lower(spec, ver=ver),
                       rd1_en=_has_src1(spec))
        op.uops_sha[ver] = s.sha(ver)
    return op


# out = in0 - in1 (elementwise); accum_out = rowmax(out)
_SUB_MAX = _register_dve_op(
    "VQ_SUB_MAX", _Spec(body=_Src0 - _Src1, accum=_AluOp.MAX))
# out = where(in0 - in1 == s0, k, s1); accum_out = rowmin(out)
# (first free-dim index where in0-in1 equals the row max s0)
_IDX_MIN = _register_dve_op(
    "VQ_IDX_MIN", _Spec(body=_select(_eq(_Src0 - _Src1, _C0), _Idx, _C1),
                        accum=_AluOp.MIN, accum_init=_C1))
# out = where(in0 == s0, k, s1); accum_out = rowmin(out)
_IDX_MIN_S = _register_dve_op(
    "VQ_IDX_MIN_S", _Spec(body=_select(_eq(_Src0, _C0), _Idx, _C1),
                          accum=_AluOp.MIN, accum_init=_C1))
# accum_out = min over k of (in1[k] if in0[k] >= s0 else s1)
_PICK_MIN = _register_dve_op(
    "VQ_PICK_MIN", _Spec(body=_select(_Src0 >= _C0, _Src1, _C1),
                         accum=_AluOp.MIN, accum_init=_C1))
# out = in0 * s0 - in1  (split residual in one pass)
_SCALE_SUB = _register_dve_op(
    "VQ_SCALE_SUB", _Spec(body=_Src0 * _C0 - _Src1))

B, L, D, H, C = 8, 2048, 512, 1024, 8192
LN_EPS = 1e-5
N_CORES = 8

TT = L // 128      # 16 token tiles
CCH = C // 512     # 16 codebook chunks
DT = D // 128      # 4 d tiles
DP = DT // 2       # 2 DoubleRow d-tile pairs
TOKC = L // 512    # 4 token chunks
SC = 2048.0        # hi-part scale (exact power of two)

F32 = mybir.dt.float32
F16 = mybir.dt.float16
F8 = mybir.dt.float8e4
I32 = mybir.dt.int32

# packed-input element offsets.  One flat f16 tensor; leading region holds
# f32 data, trailing region holds fp8 pair-layout data (both via bitcast).
OFF_X = 0                        # (L, D) f32        (f32-view offsets)
OFF_BIAS = OFF_X + L * D         # (C,) f32
NF32 = OFF_BIAS + C
OFF_G16 = 2 * NF32               # (D, C) f16        (f16 offsets)
OFF8_G8 = 2 * (OFF_G16 + D * C)  # (DP,128,2,C) f8   (f8/byte offsets)
OFF8_GL8 = OFF8_G8 + D * C       # (DP,128,2,C) f8
NTOT = (OFF8_GL8 + D * C) // 2   # total f16 elements


def build_nc():
    nc = bacc.Bacc("TRN2", target_bir_lowering=False, debug=False)

    d_f16 = nc.dram_tensor("inp", (NTOT,), F16, kind="ExternalInput")
    d_f32 = d_f16.bitcast(F32)
    d_f8 = d_f16.bitcast(F8)
    d_lab = nc.dram_tensor("labels", (128, TT), I32, kind="ExternalOutput")

    with tile.TileContext(nc) as tc:
        with tc.tile_pool(name="consts", bufs=1) as consts, \
             tc.tile_pool(name="persist", bufs=1) as persist:

            # ---------- constants ----------
            ident = consts.tile([128, 128], F32)
            make_identity(nc, ident)
            eps_t = consts.tile([128, 1], F32)
            nc.vector.memset(eps_t, LN_EPS)
            chunk_off = consts.tile([128, CCH // 2], F32)
            for j in range(CCH // 2):
                nc.vector.memset(chunk_off[:, j:j + 1], 1024.0 * j)

            # persistent split of xhat^T (d-major, 2048 scale):
            #   xh16: fp16 hi; x8l: fp8 residual pairs; x8h: fp8(xhat) pairs
            xh16 = [persist.tile([128, L], F16, name=f"xh{d}", tag=f"xh{d}")
                    for d in range(DT)]
            x8l = [persist.tile([128, 2, L], F8, name=f"x8l{j}", tag=f"x8l{j}")
                   for j in range(DP)]
            x8h = [persist.tile([128, 2, L], F8, name=f"x8h{j}", tag=f"x8h{j}")
                   for j in range(DP)]
            # per-token-tile chunk-pair winners (one col per 1024-col pair)
            gval = [persist.tile([128, CCH // 2], F32, name=f"gval{t}",
                                 tag=f"gval{t}") for t in range(TT)]
            gidx = [persist.tile([128, CCH // 2], F32, name=f"gidx{t}",
                                 tag=f"gidx{t}") for t in range(TT)]

            # ---------- phase A: LN + transpose + split ----------
            with tc.tile_pool(name="phA", bufs=1) as phA, \
                 tc.tile_pool(name="ldtmp", bufs=6) as ldtmp, \
                 tc.tile_pool(name="psTr", bufs=6, space="PSUM") as psTr:

                xnT = [phA.tile([128, L], F32, name=f"xnT{d}", tag=f"xnT{d}")
                       for d in range(DT)]
                for t in range(TT):
                    x_t = ldtmp.tile([128, D], F32, tag="x_t")
                    nc.sync.dma_start(
                        out=x_t,
                        in_=bass.AP(tensor=d_f32, offset=OFF_X + t * 128 * D,
                                    ap=[[D, 128], [1, D]]))
                    stats = ldtmp.tile([128, 6], F32, tag="stats")
                    nc.vector.bn_stats(out=stats, in_=x_t)
                    mv = ldtmp.tile([128, 2], F32, tag="mv")
                    nc.vector.bn_aggr(out=mv, in_=stats)
                    rstd = ldtmp.tile([128, 1], F32, tag="rstd")
                    nc.scalar.activation(out=rstd, in_=mv[:, 1:2],
                                         func=mybir.ActivationFunctionType.Sqrt,
                                         bias=eps_t, scale=1.0)
                    nc.vector.reciprocal(out=rstd, in_=rstd)
                    # ln weight/bias are folded into G and the bias on host
                    xn = ldtmp.tile([128, D], F32, tag="xn")
                    nc.vector.tensor_scalar(
                        out=xn, in0=x_t, scalar1=mv[:, 0:1], scalar2=rstd,
                        op0=mybir.AluOpType.subtract, op1=mybir.AluOpType.mult)
                    for d in range(DT):
                        ps_tr = psTr.tile([128, 128], F32, tag="ps_tr")
                        nc.tensor.transpose(ps_tr, xn[:, d * 128:(d + 1) * 128],
                                            ident)
                        nc.scalar.copy(out=xnT[d][:, t * 128:(t + 1) * 128],
                                       in_=ps_tr)
                    if t % 4 == 3:
                        # split this token chunk right away so phase B can
                        # start early; xh16/lo on DVE (idle here) so the ACT
                        # queue stays clear for the remaining drains
                        tksl = slice((t // 4) * 512, (t // 4 + 1) * 512)
                        for d in range(DT):
                            nc.vector.tensor_scalar(
                                out=xh16[d][:, tksl], in0=xnT[d][:, tksl],
                                scalar1=SC, scalar2=None,
                                op0=mybir.AluOpType.mult)
                            nc.scalar.activation(
                                out=x8h[d // 2][:, d % 2, tksl],
                                in_=xh16[d][:, tksl],
                                func=mybir.ActivationFunctionType.Copy,
                                scale=1.0 / SC)
                            nc.vector._custom_dve(
                                _SCALE_SUB, out=x8l[d // 2][:, d % 2, tksl],
                                in0=xnT[d][:, tksl], in1=xh16[d][:, tksl],
                                s0=SC)

            # ---------- phase B: scores + per-chunk argmax (+ inline C) ----
            with tc.tile_pool(name="cbf", bufs=3) as cbf_pool, \
                 tc.tile_pool(name="strips", bufs=2) as strips, \
                 tc.tile_pool(name="fin", bufs=2) as fin, \
                 tc.tile_pool(name="psB", bufs=4, space="PSUM") as psB:

                # chunk PAIRS: matmuls still produce 512-wide groups (one
                # PSUM bank each), but the two banks are adjacent and the
                # fused DVE argmax passes scan 1024 at once (halves the
                # per-instruction overhead on the critical DVE chain).
                NP2 = CCH // 2
                for pp in range(NP2):
                    g16 = []
                    for d in range(DT):
                        t_ = cbf_pool.tile([128, 1024], F16, name=f"g16_{d}",
                                           tag=f"g16_{d}")
                        nc.sync.dma_start(
                            out=t_,
                            in_=bass.AP(tensor=d_f16,
                                        offset=OFF_G16 + d * 128 * C + pp * 1024,
                                        ap=[[C, 128], [1, 1024]]))
                        g16.append(t_)
                    g8, gl8 = [], []
                    for j in range(DP):
                        for lst, base, nm in ((g8, OFF8_G8, "g8"),
                                              (gl8, OFF8_GL8, "gl8")):
                            t_ = cbf_pool.tile([128, 2, 1024], F8,
                                               name=f"{nm}_{j}", tag=f"{nm}_{j}")
                            nc.sync.dma_start(
                                out=t_,
                                in_=bass.AP(tensor=d_f8,
                                            offset=base + j * 256 * C + pp * 1024,
                                            ap=[[2 * C, 128], [C, 2], [1, 1024]]))
                            lst.append(t_)
                    bias_cc = cbf_pool.tile([128, 1024], F32, name="bias_cc",
                                            tag="bias_cc")
                    nc.sync.dma_start(
                        out=bias_cc,
                        in_=bass.AP(tensor=d_f32, offset=OFF_BIAS + pp * 1024,
                                    ap=[[0, 128], [1, 1024]]))

                    for t2 in range(0, TT, 2):
                        accs = []
                        for t in (t2, t2 + 1):
                            tsl = slice(t * 128, (t + 1) * 128)
                            acc = psB.tile([128, 1024], F32, tag="acc")
                            for half in range(2):
                                hsl = slice(half * 512, (half + 1) * 512)
                                for d in range(DT):
                                    nc.tensor.matmul(
                                        acc[:, hsl], lhsT=xh16[d][:, tsl],
                                        rhs=g16[d][:, hsl], start=(d == 0),
                                        stop=False)
                                for j in range(DP):
                                    nc.tensor.matmul(
                                        acc[:, hsl], lhsT=x8l[j][:, :, tsl],
                                        rhs=g8[j][:, :, hsl],
                                        perf_mode=mybir.MatmulPerfMode.DoubleRow,
                                        start=False, stop=False)
                                for j in range(DP):
                                    nc.tensor.matmul(
                                        acc[:, hsl], lhsT=x8h[j][:, :, tsl],
                                        rhs=gl8[j][:, :, hsl],
                                        perf_mode=mybir.MatmulPerfMode.DoubleRow,
                                        start=False, stop=(j == DP - 1))
                            accs.append(acc)
                        # fused DVE, batched by op type to minimize DVE-table
                        # row switches: rowmax(acc - bias) x2, then first-
                        # index-equal x2
                        sjunks = []
                        for t, acc in zip((t2, t2 + 1), accs):
                            junk = strips.tile([128, 1024], F32, tag="junk")
                            nc.vector._custom_dve(
                                _SUB_MAX, out=junk, in0=acc, in1=bias_cc,
                                accum_out=gval[t][:, pp:pp + 1])
                            sjunks.append(junk)
                        for t, sjunk in zip((t2, t2 + 1), sjunks):
                            junk2 = strips.tile([128, 1024], F32, tag="junk2")
                            nc.vector._custom_dve(
                                _IDX_MIN_S, out=junk2, in0=sjunk,
                                s0=gval[t][:, pp:pp + 1], s1=1.0e9,
                                accum_out=gidx[t][:, pp:pp + 1])

                        if pp == NP2 - 1:
                            for t in (t2, t2 + 1):
                                # phase C: combine this token tile's winners
                                gmx = fin.tile([128, 1], F32, tag="gmx")
                                nc.vector.tensor_reduce(
                                    out=gmx, in_=gval[t],
                                    axis=mybir.AxisListType.X,
                                    op=mybir.AluOpType.max)
                                cand = fin.tile([128, NP2], F32, tag="cand")
                                nc.vector.tensor_add(cand, gidx[t], chunk_off)
                                junk3 = fin.tile([128, NP2], F32, tag="junk3")
                                win = fin.tile([128, 1], F32, tag="win")
                                nc.vector._custom_dve(
                                    _PICK_MIN, out=junk3, in0=gval[t],
                                    in1=cand, s0=gmx, s1=1.0e9, accum_out=win)
                                lab = fin.tile([128, 1], I32, tag="lab")
                                nc.vector.tensor_copy(lab, win)
                                nc.sync.dma_start(out=d_lab[:, t:t + 1],
                                                  in_=lab)

    nc.compile()
    return nc


_NC_CACHE = None


def make_in_maps(input_values, ln_weight, ln_bias, proj_weight, codebook):
    input_values = np.ascontiguousarray(input_values, np.float32)
    # Fold the LN affine into the projection, then fold the projection into
    # the codebook:  t.c = xhat.(P'^T cb) with P' = P diag(g); the constant
    # P b shifts every score by (P b).c, folded into the bias (fp64, exact).
    pw64 = proj_weight.astype(np.float64) \
        * np.asarray(ln_weight, np.float64)[None, :]
    pb = proj_weight.astype(np.float64) @ np.asarray(ln_bias, np.float64)
    cb64 = codebook.astype(np.float64)
    G = pw64.T @ cb64                                     # (D, C)
    bias = (SC * (0.5 * (cb64 ** 2).sum(0) - pb @ cb64)).astype(np.float32)

    G16 = G.astype(np.float16)
    G8 = G16.astype(_F8NP)
    Gl8 = ((G - G16.astype(np.float64)) * SC).astype(np.float32).astype(_F8NP)
    # DoubleRow pair layout: [pair j, partition p, slot s, c] where slot s
    # holds d-tile 2j+s (d = (2j+s)*128 + p)
    G8p = np.ascontiguousarray(G8.reshape(DP, 2, 128, C).transpose(0, 2, 1, 3))
    Gl8p = np.ascontiguousarray(
        Gl8.reshape(DP, 2, 128, C).transpose(0, 2, 1, 3))

    tail16 = np.concatenate([
        G16.ravel(),
        G8p.reshape(-1).view(np.uint8).view(np.float16),
        Gl8p.reshape(-1).view(np.uint8).view(np.float16),
    ])

    in_maps = []
    for i in range(N_CORES):
        head32 = np.concatenate([input_values[i].ravel(),
                                 bias.ravel()]).astype(np.float32)
        blob = np.concatenate([head32.view(np.float16), tail16])
        in_maps.append({"inp": np.ascontiguousarray(blob, np.float16)})
    return in_maps


def kernel(input_values, ln_weight, ln_bias, proj_weight, codebook):
    global _NC_CACHE
    if _NC_CACHE is None:
        _NC_CACHE = build_nc()
    nc = _NC_CACHE

    in_maps = make_in_maps(input_values, ln_weight, ln_bias, proj_weight,
                           codebook)
    res = run_bass_kernel_spmd(nc, in_maps, core_ids=list(range(N_CORES)))
    out = np.empty((B, L), np.int32)
    for i in range(N_CORES):
        out[i] = res.results[i]["labels"].T.reshape(L)
    return out


# revision 81
# speedup vs baseline: 1.2010x; 1.1757x over previous
"""Trainium2 Bass kernel for BestRQ vector-quantization codebook lookup.

Key algebraic move: the projection never materializes on chip.  Scores
satisfy  t.c = (P xhat).c = xhat.(P^T cb) = xhat.G  with G = P^T cb
precomputed on host in fp64 (LN affine folded into P and the bias), so the
contraction shrinks from H=1024 to D=512 and the whole on-chip projection
phase disappears.

Per NeuronCore (data-parallel over batch):
  x (2048,512) --LayerNorm--> xhat --PE transpose--> xnT (d-major)
  x split (2048 scale): xh = fp16(xhat*2048); fp8 pairs xl8 (residual),
  xh8 (fp8(xhat)) in DoubleRow pair layout.
  G split on host: G16 = fp16(G); G8 = fp8(G16); Gl8 = fp8(2048*(G-G16)),
  fp8 parts in DoubleRow pair layout; bias = 2048*(0.5||c||^2 - (P b).c).
  score*2048 = xh@G16  (4 fp16 matmuls, 1 cyc/row)
             + xl8(x)G8 + xh8(x)Gl8  (4 DoubleRow fp8 matmuls, 0.5 cyc/row)
  argmax via two fused custom DVE ops per (1024-col chunk pair,
  token-tile): rowmax(acc - bias) then first-index-equal; the 8-pair
  combine runs inline after each token tile's last pair.

Numerics: G carried to ~2^-15, xhat to ~2^-15; measured on hardware:
1 / 16384 label flips vs the fp32 reference (rel err 8.8e-4, gate 2e-2).

Dispatch-overhead notes: every PJRT argument costs ~56us per call on this
axon setup, so all inputs ship in ONE flat f16 tensor (f32/f8 regions are
read through bitcast views).
"""

import numpy as np

import concourse.bacc as bacc
import concourse.bass as bass
import concourse.mybir as mybir
import concourse.tile as tile
from concourse import dve_ops as _dvo
from concourse.bass_utils import run_bass_kernel_spmd
from concourse.dve_spec import (AluOp as _AluOp, C0 as _C0, C1 as _C1,
                                Idx as _Idx, Spec as _Spec,
                                Src0 as _Src0, Src1 as _Src1, _has_src1,
                                eq as _eq, lower as _lower,
                                select as _select)
from concourse.dve_uop import DveOpSpec as _DveOpSpec
from concourse.masks import make_identity

import ml_dtypes
_F8NP = ml_dtypes.float8_e4m3fn


def _register_dve_op(name, spec):
    """Register a custom fused DVE op (documented extension point: the uop
    program is compiled into the per-NEFF DVE table; sha self-pinned)."""
    for op in _dvo.OPS:
        if op.name == name:
            return op
    op = _dvo.DveOp(name, spec, subdim=False, uops_sha={})
    _dvo.OPS.append(op)
    _dvo.CUSTOM_DVE_SPECS[name] = spec
    opcode = _dvo._CUSTOM_DVE_ROW_BASE + len(_dvo.OPS) - 1
    _dvo._SUB_OPCODE_FOR_NAME[name] = opcode
    for ver in ("v3", "v4"):
        s = _DveOpSpec(name=name, opcode=opcode, uops=_lower(spec, ver=ver),
                       rd1_en=_has_src1(spec))
        op.uops_sha[ver] = s.sha(ver)
    return op


# out = in0 - in1 (elementwise); accum_out = rowmax(out)
_SUB_MAX = _register_dve_op(
    "VQ_SUB_MAX", _Spec(body=_Src0 - _Src1, accum=_AluOp.MAX))
# out = where(in0 - in1 == s0, k, s1); accum_out = rowmin(out)
# (first free-dim index where in0-in1 equals the row max s0)
_IDX_MIN = _register_dve_op(
    "VQ_IDX_MIN", _Spec(body=_select(_eq(_Src0 - _Src1, _C0), _Idx, _C1),
                        accum=_AluOp.MIN, accum_init=_C1))
# out = where(in0 == s0, k, s1); accum_out = rowmin(out)
_IDX_MIN_S = _register_dve_op(
    "VQ_IDX_MIN_S", _Spec(body=_select(_eq(_Src0, _C0), _Idx, _C1),
                          accum=_AluOp.MIN, accum_init=_C1))
# accum_out = min over k of (in1[k] if in0[k] >= s0 else s1)
_PICK_MIN = _register_dve_op(
    "VQ_PICK_MIN", _Spec(body=_select(_Src0 >= _C0, _Src1, _C1),
                         accum=_AluOp.MIN, accum_init=_C1))
# out = in0 * s0 - in1  (split residual in one pass)
_SCALE_SUB = _register_dve_op(
    "VQ_SCALE_SUB", _Spec(body=_Src0 * _C0 - _Src1))

B, L, D, H, C = 8, 2048, 512, 1024, 8192
LN_EPS = 1e-5
N_CORES = 8

TT = L // 128      # 16 token tiles
CCH = C // 512     # 16 codebook chunks
DT = D // 128      # 4 d tiles
DP = DT // 2       # 2 DoubleRow d-tile pairs
TOKC = L // 512    # 4 token chunks
SC = 2048.0        # hi-part scale (exact power of two)

F32 = mybir.dt.float32
F16 = mybir.dt.float16
F8 = mybir.dt.float8e4
I32 = mybir.dt.int32

# packed-input element offsets.  One flat f16 tensor; leading region holds
# f32 data, trailing region holds fp8 pair-layout data (both via bitcast).
OFF_X = 0                        # (L, D) f32        (f32-view offsets)
OFF_BIAS = OFF_X + L * D         # (C,) f32
NF32 = OFF_BIAS + C
OFF_G16 = 2 * NF32               # (D, C) f16        (f16 offsets)
OFF8_G8 = 2 * (OFF_G16 + D * C)  # (DP,128,2,C) f8   (f8/byte offsets)
OFF8_GL8 = OFF8_G8 + D * C       # (DP,128,2,C) f8
NTOT = (OFF8_GL8 + D * C) // 2   # total f16 elements


def build_nc():
    nc = bacc.Bacc("TRN2", target_bir_lowering=False, debug=False)

    d_f16 = nc.dram_tensor("inp", (NTOT,), F16, kind="ExternalInput")
    d_f32 = d_f16.bitcast(F32)
    d_f8 = d_f16.bitcast(F8)
    d_lab = nc.dram_tensor("labels", (128, TT), I32, kind="ExternalOutput")

    with tile.TileContext(nc) as tc:
        with tc.tile_pool(name="consts", bufs=1) as consts, \
             tc.tile_pool(name="persist", bufs=1) as persist:

            # ---------- constants ----------
            ident = consts.tile([128, 128], F32)
            make_identity(nc, ident)
            eps_t = consts.tile([128, 1], F32)
            nc.vector.memset(eps_t, LN_EPS)
            chunk_off = consts.tile([128, CCH // 2], F32)
            for j in range(CCH // 2):
                nc.vector.memset(chunk_off[:, j:j + 1], 1024.0 * j)

            # persistent split of xhat^T (d-major, 2048 scale):
            #   xh16: fp16 hi; x8l: fp8 residual pairs; x8h: fp8(xhat) pairs
            xh16 = [persist.tile([128, L], F16, name=f"xh{d}", tag=f"xh{d}")
                    for d in range(DT)]
            x8l = [persist.tile([128, 2, L], F8, name=f"x8l{j}", tag=f"x8l{j}")
                   for j in range(DP)]
            x8h = [persist.tile([128, 2, L], F8, name=f"x8h{j}", tag=f"x8h{j}")
                   for j in range(DP)]
            # per-token-tile chunk-pair winners (one col per 1024-col pair)
            gval = [persist.tile([128, CCH // 2], F32, name=f"gval{t}",
                                 tag=f"gval{t}") for t in range(TT)]
            gidx = [persist.tile([128, CCH // 2], F32, name=f"gidx{t}",
                                 tag=f"gidx{t}") for t in range(TT)]

            # ---------- phase A: LN + transpose + split ----------
            with tc.tile_pool(name="phA", bufs=1) as phA, \
                 tc.tile_pool(name="ldtmp", bufs=6) as ldtmp, \
                 tc.tile_pool(name="psTr", bufs=6, space="PSUM") as psTr:

                xnT = [phA.tile([128, L], F32, name=f"xnT{d}", tag=f"xnT{d}")
                       for d in range(DT)]
                for t in range(TT):
                    x_t = ldtmp.tile([128, D], F32, tag="x_t")
                    nc.sync.dma_start(
                        out=x_t,
                        in_=bass.AP(tensor=d_f32, offset=OFF_X + t * 128 * D,
                                    ap=[[D, 128], [1, D]]))
                    stats = ldtmp.tile([128, 6], F32, tag="stats")
                    nc.vector.bn_stats(out=stats, in_=x_t)
                    mv = ldtmp.tile([128, 2], F32, tag="mv")
                    nc.vector.bn_aggr(out=mv, in_=stats)
                    rstd = ldtmp.tile([128, 1], F32, tag="rstd")
                    nc.scalar.activation(out=rstd, in_=mv[:, 1:2],
                                         func=mybir.ActivationFunctionType.Sqrt,
                                         bias=eps_t, scale=1.0)
                    nc.vector.reciprocal(out=rstd, in_=rstd)
                    # ln weight/bias are folded into G and the bias on host
                    xn = ldtmp.tile([128, D], F32, tag="xn")
                    nc.vector.tensor_scalar(
                        out=xn, in0=x_t, scalar1=mv[:, 0:1], scalar2=rstd,
                        op0=mybir.AluOpType.subtract, op1=mybir.AluOpType.mult)
                    for d in range(DT):
                        ps_tr = psTr.tile([128, 128], F32, tag="ps_tr")
                        nc.tensor.transpose(ps_tr, xn[:, d * 128:(d + 1) * 128],
                                            ident)
                        nc.scalar.copy(out=xnT[d][:, t * 128:(t + 1) * 128],
                                       in_=ps_tr)
                    if t % 4 == 3:
                        # split this token chunk right away so phase B can
                        # start early; xh16/lo on DVE (idle here) so the ACT
                        # queue stays clear for the remaining drains
                        tksl = slice((t // 4) * 512, (t // 4 + 1) * 512)
                        for d in range(DT):
                            nc.vector.tensor_scalar(
                                out=xh16[d][:, tksl], in0=xnT[d][:, tksl],
                                scalar1=SC, scalar2=None,
                                op0=mybir.AluOpType.mult)
                            nc.scalar.activation(
                                out=x8h[d // 2][:, d % 2, tksl],
                                in_=xh16[d][:, tksl],
                                func=mybir.ActivationFunctionType.Copy,
                                scale=1.0 / SC)
                            nc.vector._custom_dve(
                                _SCALE_SUB, out=x8l[d // 2][:, d % 2, tksl],
                                in0=xnT[d][:, tksl], in1=xh16[d][:, tksl],
                                s0=SC)

            # ---------- phase B: scores + per-chunk argmax (+ inline C) ----
            with tc.tile_pool(name="cbf", bufs=3) as cbf_pool, \
                 tc.tile_pool(name="strips", bufs=2) as strips, \
                 tc.tile_pool(name="fin", bufs=2) as fin, \
                 tc.tile_pool(name="psB", bufs=4, space="PSUM") as psB:

                # chunk PAIRS: matmuls still produce 512-wide groups (one
                # PSUM bank each), but the two banks are adjacent and the
                # fused DVE argmax passes scan 1024 at once (halves the
                # per-instruction overhead on the critical DVE chain).
                NP2 = CCH // 2
                for pp in range(NP2):
                    g16 = []
                    for d in range(DT):
                        t_ = cbf_pool.tile([128, 1024], F16, name=f"g16_{d}",
                                           tag=f"g16_{d}")
                        nc.sync.dma_start(
                            out=t_,
                            in_=bass.AP(tensor=d_f16,
                                        offset=OFF_G16 + d * 128 * C + pp * 1024,
                                        ap=[[C, 128], [1, 1024]]))
                        g16.append(t_)
                    g8, gl8 = [], []
                    for j in range(DP):
                        for lst, base, nm in ((g8, OFF8_G8, "g8"),
                                              (gl8, OFF8_GL8, "gl8")):
                            t_ = cbf_pool.tile([128, 2, 1024], F8,
                                               name=f"{nm}_{j}", tag=f"{nm}_{j}")
                            nc.sync.dma_start(
                                out=t_,
                                in_=bass.AP(tensor=d_f8,
                                            offset=base + j * 256 * C + pp * 1024,
                                            ap=[[2 * C, 128], [C, 2], [1, 1024]]))
                            lst.append(t_)
                    bias_cc = cbf_pool.tile([128, 1024], F32, name="bias_cc",
                                            tag="bias_cc")
                    nc.sync.dma_start(
                        out=bias_cc,
                        in_=bass.AP(tensor=d_f32, offset=OFF_BIAS + pp * 1024,
                                    ap=[[0, 128], [1, 1024]]))

                    for t2 in range(0, TT, 2):
                        accs = []
                        for t in (t2, t2 + 1):
                            tsl = slice(t * 128, (t + 1) * 128)
                            acc = psB.tile([128, 1024], F32, tag="acc")
                            for half in range(2):
                                hsl = slice(half * 512, (half + 1) * 512)
                                for d in range(DT):
                                    nc.tensor.matmul(
                                        acc[:, hsl], lhsT=xh16[d][:, tsl],
                                        rhs=g16[d][:, hsl], start=(d == 0),
                                        stop=False)
                                for j in range(DP):
                                    nc.tensor.matmul(
                                        acc[:, hsl], lhsT=x8l[j][:, :, tsl],
                                        rhs=g8[j][:, :, hsl],
                                        perf_mode=mybir.MatmulPerfMode.DoubleRow,
                                        start=False, stop=False)
                                for j in range(DP):
                                    nc.tensor.matmul(
                                        acc[:, hsl], lhsT=x8h[j][:, :, tsl],
                                        rhs=gl8[j][:, :, hsl],
                                        perf_mode=mybir.MatmulPerfMode.DoubleRow,
                                        start=False, stop=(j == DP - 1))
                            accs.append(acc)
                        # fused DVE, batched by op type to minimize DVE-table
                        # row switches: rowmax(acc - bias) x2, then first-
                        # index-equal x2
                        sjunks = []
                        for t, acc in zip((t2, t2 + 1), accs):
                            junk = strips.tile([128, 1024], F32, tag="junk")
                            nc.vector._custom_dve(
                                _SUB_MAX, out=junk, in0=acc, in1=bias_cc,
                                accum_out=gval[t][:, pp:pp + 1])
                            sjunks.append(junk)
                        for t, sjunk in zip((t2, t2 + 1), sjunks):
                            junk2 = strips.tile([128, 1024], F32, tag="junk2")
                            nc.vector._custom_dve(
                                _IDX_MIN_S, out=junk2, in0=sjunk,
                                s0=gval[t][:, pp:pp + 1], s1=1.0e9,
                                accum_out=gidx[t][:, pp:pp + 1])

                        if pp == NP2 - 1:
                            for t in (t2, t2 + 1):
                                # phase C: combine this token tile's winners
                                gmx = fin.tile([128, 1], F32, tag="gmx")
                                nc.vector.tensor_reduce(
                                    out=gmx, in_=gval[t],
                                    axis=mybir.AxisListType.X,
                                    op=mybir.AluOpType.max)
                                cand = fin.tile([128, NP2], F32, tag="cand")
                                nc.vector.tensor_add(cand, gidx[t], chunk_off)
                                junk3 = fin.tile([128, NP2], F32, tag="junk3")
                                win = fin.tile([128, 1], F32, tag="win")
                                nc.vector._custom_dve(
                                    _PICK_MIN, out=junk3, in0=gval[t],
                                    in1=cand, s0=gmx, s1=1.0e9, accum_out=win)
                                lab = fin.tile([128, 1], I32, tag="lab")
                                nc.vector.tensor_copy(lab, win)
                                nc.sync.dma_start(out=d_lab[:, t:t + 1],
                                                  in_=lab)

    nc.compile()
    return nc


_NC_CACHE = None


def make_in_maps(input_values, ln_weight, ln_bias, proj_weight, codebook):
    input_values = np.ascontiguousarray(input_values, np.float32)
    # Fold the LN affine into the projection, then fold the projection into
    # the codebook:  t.c = xhat.(P'^T cb) with P' = P diag(g); the constant
    # P b shifts every score by (P b).c, folded into the bias (fp64, exact).
    pw64 = proj_weight.astype(np.float64) \
        * np.asarray(ln_weight, np.float64)[None, :]
    pb = proj_weight.astype(np.float64) @ np.asarray(ln_bias, np.float64)
    cb64 = codebook.astype(np.float64)
    G = pw64.T @ cb64                                     # (D, C)
    bias = (SC * (0.5 * (cb64 ** 2).sum(0) - pb @ cb64)).astype(np.float32)

    G16 = G.astype(np.float16)
    G8 = G16.astype(_F8NP)
    Gl8 = ((G - G16.astype(np.float64)) * SC).astype(np.float32).astype(_F8NP)
    # DoubleRow pair layout: [pair j, partition p, slot s, c] where slot s
    # holds d-tile 2j+s (d = (2j+s)*128 + p)
    G8p = np.ascontiguousarray(G8.reshape(DP, 2, 128, C).transpose(0, 2, 1, 3))
    Gl8p = np.ascontiguousarray(
        Gl8.reshape(DP, 2, 128, C).transpose(0, 2, 1, 3))

    tail16 = np.concatenate([
        G16.ravel(),
        G8p.reshape(-1).view(np.uint8).view(np.float16),
        Gl8p.reshape(-1).view(np.uint8).view(np.float16),
    ])

    in_maps = []
    for i in range(N_CORES):
        head32 = np.concatenate([input_values[i].ravel(),
                                 bias.ravel()]).astype(np.float32)
        blob = np.concatenate([head32.view(np.float16), tail16])
        in_maps.append({"inp": np.ascontiguousarray(blob, np.float16)})
    return in_maps


def kernel(input_values, ln_weight, ln_bias, proj_weight, codebook):
    global _NC_CACHE
    if _NC_CACHE is None:
        _NC_CACHE = build_nc()
    nc = _NC_CACHE

    in_maps = make_in_maps(input_values, ln_weight, ln_bias, proj_weight,
                           codebook)
    res = run_bass_kernel_spmd(nc, in_maps, core_ids=list(range(N_CORES)))
    out = np.empty((B, L), np.int32)
    for i in range(N_CORES):
        out[i] = res.results[i]["labels"].T.reshape(L)
    return out
